# revision 15
# baseline (speedup 1.0000x reference)
"""Trainium2 Bass kernel for the DAN classifier (gather + segment-mean + MLP + BCE).

Data-parallel across 8 NeuronCores: each core owns 512 whole sentences
(segments). Host splits the token stream at sentence boundaries, buckets each
core's tokens by (segment-group of 128, vocab-slice) so embedding row indices
fit in int16 for the bulk dma_gather, and pads each bucket to a common
capacity. On device: dma_gather (4 parallel SWDGE queues, one per vocab
slice - each queue's descriptors are generated by a different pair of Q7
cores) pulls bf16 embedding rows from HBM; a one-hot(segment) matmul on the
TensorEngine accumulates per-segment sums in fp32 PSUM; the tiny MLP head +
BCE run on-chip; each core emits its partial loss. Host sums the 8 partials
(the all-reduce of the scalar loss).

Key tunings over the first working version (guided by perfetto traces):
- vocab slices are uneven: queue 0 (whose gather instructions carry a fixed
  extra ~6-7us of drain/overhead per instruction) gets a smaller share.
- gather sizes are quantized per group for queues 1-3 so consecutive gather
  instructions share the num_idxs register (register reuse forced ~5us
  dispatch stalls between groups).
- BCE tail uses Softplus directly (loss = sum sp(x) - y*x), removing the
  Exp/Ln activations and a 1.5us activation-table load from the critical
  path; the y*x reduction is a single fused scalar_tensor_tensor.
- the last (small) group's one-hots are prebuilt during the gather phase.
- constant/metadata loads are consolidated into 3 DMAs (int16 idx, bf16
  blob, fp32 blob) instead of ~19 serial ~700ns HWDGE transfers.
"""

import sys

try:
    import concourse  # noqa: F401
except ImportError:
    sys.path.insert(0, "/opt/trn_rl_repo")

import ml_dtypes
import numpy as np

import concourse.tile as tile
from concourse import bacc, mybir
from concourse.bass_utils import run_bass_kernel_spmd

# Problem constants (hardcoded per harness contract).
V = 100000
H = 128
B = 4096
T = 409600
N_CORES = 8

SEGS_PER_CORE = B // N_CORES          # 512
# Tapered segment groups: early groups big (pipeline fill), last group small
# so the final gather + matmul + MLP tail is short.
GROUP_SEGS = (128, 128, 128, 96, 32)
GROUP_STARTS = (0, 128, 256, 384, 480)
N_GROUPS = len(GROUP_SEGS)
N_QUARTERS = 4                        # vocab slices / SWDGE queues
# Uneven vocab slices: queue 0 smaller (fixed per-instruction overhead there),
# queues 1-3 equal. All slice sizes must stay < 32768 for int16 indices.
QB = (0, 20608, 47072, 73536, 100000)
QSIZES = tuple(QB[i + 1] - QB[i] for i in range(N_QUARTERS))

F32 = mybir.dt.float32
BF16 = mybir.dt.bfloat16
I16 = mybir.dt.int16
BF16_NP = ml_dtypes.bfloat16


def _build(nc, c_sub, tiles_sb):
    """Emit the SPMD per-core graph. c_sub = padded tokens per sub-block."""
    n_tiles = c_sub // 128  # token tiles of 128 per sub-block
    n_sub = N_GROUPS * N_QUARTERS
    c16 = c_sub // 16

    # fp32 blob layout (columns): ident(128) | w_hid(128) | b_hid(1) |
    # w_out(1) | recip(N_GROUPS) | y(SEGS_PER_CORE, row 0) | b_out(1, row 0)
    O_IDENT = 0
    O_WHID = O_IDENT + 128
    O_BHID = O_WHID + H
    O_WOUT = O_BHID + 1
    O_RECIP = O_WOUT + 1
    O_Y = O_RECIP + N_GROUPS
    O_BOUT = O_Y + SEGS_PER_CORE
    F32_COLS = O_BOUT + 1

    # bf16 blob layout: iota(128) | seg(n_sub * n_tiles)
    O_IOTA = 0
    O_SEG = O_IOTA + 128
    BF16_COLS = O_SEG + n_sub * n_tiles

    # ---- DRAM parameters (per-core shards arrive via in_maps) ----
    embed = nc.dram_tensor("embed", [V, H], BF16, kind="ExternalInput")
    idx_d = nc.dram_tensor("idx", [128, n_sub * c16], I16, kind="ExternalInput")
    fblob_d = nc.dram_tensor("fblob", [128, F32_COLS], F32, kind="ExternalInput")
    bblob_d = nc.dram_tensor("bblob", [128, BF16_COLS], BF16,
                             kind="ExternalInput")
    out_d = nc.dram_tensor("out", [1, 1], F32, kind="ExternalOutput")

    with tile.TileContext(nc) as tc:
        with (
            tc.tile_pool(name="const", bufs=1) as cpool,
            tc.tile_pool(name="gather", bufs=4) as gpool,
            tc.tile_pool(name="onehot", bufs=4) as opool,
            tc.tile_pool(name="mlp", bufs=1) as mpool,
            tc.tile_pool(name="psum", bufs=2, space="PSUM") as ppool,
            tc.tile_pool(name="psum_mlp", bufs=1, space="PSUM") as pmpool,
        ):
            # ---- warm activation tables first ----
            warm = cpool.tile([1, 1], F32)
            for fn in (mybir.ActivationFunctionType.Tanh,
                       mybir.ActivationFunctionType.Exp,
                       mybir.ActivationFunctionType.Ln):
                nc.scalar.activation(out=warm[:], in_=warm[:], func=fn)

            # ---- consolidated constants / metadata loads ----
            idx_sb = cpool.tile([128, n_sub * c16], I16)
            # group 0's idx slice first (gates the first gathers), rest after
            g0q = N_QUARTERS * c16
            nc.sync.dma_start(out=idx_sb[:, :g0q], in_=idx_d[:, :g0q])
            nc.sync.dma_start(out=idx_sb[:, g0q:], in_=idx_d[:, g0q:])
            fblob = cpool.tile([128, F32_COLS], F32)
            nc.sync.dma_start(out=fblob[:], in_=fblob_d[:])
            bblob = cpool.tile([128, BF16_COLS], BF16)
            nc.sync.dma_start(out=bblob[:], in_=bblob_d[:])

            ident_sb = fblob[:, O_IDENT:O_IDENT + 128]
            w_hid_sb = fblob[:, O_WHID:O_WHID + H]
            b_hid_sb = fblob[:, O_BHID:O_BHID + 1]
            w_out_sb = fblob[:, O_WOUT:O_WOUT + 1]
            recip_sb = fblob[:, O_RECIP:O_RECIP + N_GROUPS]
            y_sb = fblob[0:1, O_Y:O_Y + SEGS_PER_CORE]
            b_out_sb = fblob[0:1, O_BOUT:O_BOUT + 1]
            iota_sb = bblob[:, O_IOTA:O_IOTA + 128]
            seg_sb = bblob[:, O_SEG:]

            sent_t = mpool.tile([128, SEGS_PER_CORE], F32)  # [H, seg]
            psum_hid = pmpool.tile([128, SEGS_PER_CORE], F32, tag="psum_hid")
            psum_p = pmpool.tile([1, SEGS_PER_CORE], F32, tag="psum_p")
            hid = mpool.tile([128, SEGS_PER_CORE], F32)

            # ---- prebuild the last two groups' one-hots (persistent tiles;
            # they would otherwise sit on the critical tail waiting for
            # onehot-pool buffers chained to earlier groups' gathers) ----
            prebuilt = {}
            for g in (N_GROUPS - 2, N_GROUPS - 1):
                for q in range(N_QUARTERS):
                    sb = g * N_QUARTERS + q
                    tsb = tiles_sb[sb]
                    oh = cpool.tile([128, tsb, 128], BF16,
                                    name=f"ohp_{g}_{q}")
                    nc.vector.tensor_tensor(
                        out=oh[:],
                        in0=seg_sb[:, sb * n_tiles : sb * n_tiles + tsb]
                        .rearrange("p (t u) -> p t u", u=1)
                        .to_broadcast([128, tsb, 128]),
                        in1=iota_sb
                        .rearrange("p (u m) -> p u m", u=1)
                        .to_broadcast([128, tsb, 128]),
                        op=mybir.AluOpType.is_equal,
                    )
                    prebuilt[sb] = oh

            # chunk size: 6 tiles = 768 idxs = 48 descriptors per DMA engine
            # per instruction, below the 64-desc SDMA packet ceiling so
            # single_packet (whole-instruction packet concat, which amortizes
            # the ~27% per-packet bus overhead at 256B descriptors) is legal.
            CH = 6
            for g in range(N_GROUPS):
                gt = gpool.tile([128, N_QUARTERS, n_tiles, 128], BF16,
                                tag="gather")
                gmax = max(tiles_sb[g * N_QUARTERS + q]
                           for q in range(N_QUARTERS))
                for lo in range(0, gmax, CH):
                    for q in (1, 2, 3, 0):
                        sb = g * N_QUARTERS + q
                        hi = min(lo + CH, tiles_sb[sb])
                        if hi <= lo:
                            continue
                        nidx = (hi - lo) * 128
                        nc.gpsimd.dma_gather(
                            gt[:, q, lo:hi, :],
                            embed[QB[q] : QB[q] + QSIZES[q], :],
                            idx_sb[:, sb * c16 + lo * 8 : sb * c16 + lo * 8
                                   + nidx // 16],
                            nidx,
                            nidx,
                            H,
                            single_packet=True,
                            queue_num=q,
                        )

                psum_s = ppool.tile([128, H], F32, tag="psum_s")
                for q in range(N_QUARTERS):
                    sb = g * N_QUARTERS + q
                    tsb = tiles_sb[sb]
                    if sb in prebuilt:
                        oh = prebuilt[sb]
                    else:
                        oh = opool.tile([128, n_tiles, 128], BF16, tag="onehot")
                        nc.vector.tensor_tensor(
                            out=oh[:, :tsb, :],
                            in0=seg_sb[:, sb * n_tiles : sb * n_tiles + tsb]
                            .rearrange("p (t u) -> p t u", u=1)
                            .to_broadcast([128, tsb, 128]),
                            in1=iota_sb
                            .rearrange("p (u m) -> p u m", u=1)
                            .to_broadcast([128, tsb, 128]),
                            op=mybir.AluOpType.is_equal,
                        )
                    for j in range(tsb):
                        nc.tensor.matmul(
                            psum_s[:],
                            lhsT=oh[:, j, :],
                            rhs=gt[:, q, j, :],
                            start=(q == 0 and j == 0),
                            stop=(q == N_QUARTERS - 1 and j == tsb - 1),
                        )

                # segment means for this group: psum * (1/count)
                gstart, gsize = GROUP_STARTS[g], GROUP_SEGS[g]
                sent_g = mpool.tile([128, H], F32, tag="sent_g")
                nc.vector.tensor_scalar(
                    out=sent_g[:],
                    in0=psum_s[:],
                    scalar1=recip_sb[:, g : g + 1],
                    scalar2=None,
                    op0=mybir.AluOpType.mult,
                )
                # transpose [seg, H] -> [H, seg] chunk of sent_t
                psum_t = ppool.tile([128, 128], F32, tag="psum_t")
                nc.tensor.transpose(psum_t[:], sent_g[:], ident_sb)
                nc.vector.tensor_copy(
                    out=sent_t[:, gstart : gstart + gsize],
                    in_=psum_t[:, :gsize],
                )
                nc.tensor.matmul(psum_hid[:, gstart : gstart + gsize],
                                 lhsT=w_hid_sb,
                                 rhs=sent_t[:, gstart : gstart + gsize],
                                 start=True, stop=True)
                nc.scalar.activation(
                    out=hid[:, gstart : gstart + gsize],
                    in_=psum_hid[:, gstart : gstart + gsize],
                    func=mybir.ActivationFunctionType.Tanh,
                    bias=b_hid_sb,
                )
                nc.tensor.matmul(psum_p[:, gstart : gstart + gsize],
                                 lhsT=w_out_sb,
                                 rhs=hid[:, gstart : gstart + gsize],
                                 start=True, stop=True)

            # ---- BCE tail ----
            # x = W_out^T hid + b_out ;  loss = sum softplus(x) - sum y*x
            # softplus(x) = ln(1 + exp(x)) via the Exp and Ln tables.
            ep = mpool.tile([1, SEGS_PER_CORE], F32)
            nc.scalar.activation(
                out=ep[:], in_=psum_p[:],
                func=mybir.ActivationFunctionType.Exp,
                bias=b_out_sb,
            )
            sp = mpool.tile([1, SEGS_PER_CORE], F32)
            sp_sum = mpool.tile([1, 1], F32)
            nc.scalar.activation(
                out=sp[:], in_=ep[:],
                func=mybir.ActivationFunctionType.Ln,
                bias=1.0, accum_out=sp_sum[:],
            )
            yx = mpool.tile([1, SEGS_PER_CORE], F32)
            yx_sum = mpool.tile([1, 1], F32)
            nc.vector.scalar_tensor_tensor(
                out=yx[:], in0=psum_p[:], scalar=b_out_sb, in1=y_sb,
                op0=mybir.AluOpType.add, op1=mybir.AluOpType.mult,
                accum_out=yx_sum[:],
            )
            loss = mpool.tile([1, 1], F32)
            nc.vector.tensor_tensor(out=loss[:], in0=sp_sum[:], in1=yx_sum[:],
                                    op=mybir.AluOpType.subtract)
            nc.sync.dma_start(out=out_d[:], in_=loss[:])

    nc.compile()
    return nc


def _prep_inputs(token_ids, segment_ids, y_true, embed_table, W_hid, b_hid,
                 W_out, b_out):
    """Host-side shard + bucket + pad. Returns (c_sub, tiles_sb, in_maps)."""
    token_ids = np.asarray(token_ids, dtype=np.int64)
    segment_ids = np.asarray(segment_ids, dtype=np.int64)
    y_true = np.asarray(y_true, dtype=np.float32)
    embed_bf16 = np.ascontiguousarray(
        np.asarray(embed_table, dtype=np.float32).astype(BF16_NP))

    # sentence-aligned core boundaries
    bounds = np.searchsorted(segment_ids, np.arange(0, B + 1, SEGS_PER_CORE))
    counts = np.bincount(segment_ids, minlength=B).astype(np.float32)
    recip_all = 1.0 / np.maximum(counts, 1.0)

    qb_arr = np.asarray(QB[1:-1], dtype=np.int64)

    # bucket tokens per (core, group, vocab-slice)
    per_core = []
    for c in range(N_CORES):
        lo, hi = bounds[c], bounds[c + 1]
        tid = token_ids[lo:hi]
        seg_loc = segment_ids[lo:hi] - c * SEGS_PER_CORE
        starts = np.asarray(GROUP_STARTS, dtype=np.int64)
        grp = np.searchsorted(starts[1:], seg_loc, side="right")
        seg_in_grp = (seg_loc - starts[grp]).astype(np.float32)
        q = np.searchsorted(qb_arr, tid, side="right")
        loc_idx = (tid - np.asarray(QB, dtype=np.int64)[q]).astype(np.int64)
        subs = []
        for g in range(N_GROUPS):
            for qq in range(N_QUARTERS):
                sel = (grp == g) & (q == qq)
                li, sg = loc_idx[sel], seg_in_grp[sel]
                order = np.argsort(li, kind="stable")
                subs.append((li[order], sg[order]))
        per_core.append(subs)

    n_sub = N_GROUPS * N_QUARTERS
    sb_max = [0] * n_sub
    for c in range(N_CORES):
        for sbi, (li, sg) in enumerate(per_core[c]):
            sb_max[sbi] = max(sb_max[sbi], li.shape[0])
    # Quantize gather sizes: queues 1-3 share one tile count per group (so
    # consecutive gather instructions reuse the same num_idxs register);
    # queue 0 gets its own (smaller) count per group.
    tiles_raw = [(m + 127) // 128 for m in sb_max]
    tiles_sb = [0] * n_sub
    for g in range(N_GROUPS):
        t123 = max(tiles_raw[g * N_QUARTERS + q] for q in (1, 2, 3))
        for q in range(N_QUARTERS):
            sb = g * N_QUARTERS + q
            tiles_sb[sb] = tiles_raw[sb] if q == 0 else t123
    tiles_sb = tuple(tiles_sb)
    c_sub = 128 * max(tiles_sb)
    n_tiles = c_sub // 128
    c16 = c_sub // 16

    # fp32 / bf16 blob column offsets (keep in lockstep with _build)
    O_WHID = 128
    O_BHID = O_WHID + H
    O_WOUT = O_BHID + 1
    O_RECIP = O_WOUT + 1
    O_Y = O_RECIP + N_GROUPS
    O_BOUT = O_Y + SEGS_PER_CORE
    F32_COLS = O_BOUT + 1
    BF16_COLS = 128 + n_sub * n_tiles

    iota = np.broadcast_to(np.arange(128, dtype=np.float32), (128, 128))
    w_hid_f = np.ascontiguousarray(np.asarray(W_hid, dtype=np.float32))
    b_hid_f = np.asarray(b_hid, dtype=np.float32).reshape(H)
    w_out_f = np.asarray(W_out, dtype=np.float32).reshape(H)
    b_out_f = np.asarray(b_out, dtype=np.float32).reshape(1)

    in_maps = []
    for c in range(N_CORES):
        idx_arr = np.zeros((128, n_sub * c16), dtype=np.int16)
        seg_arr = np.full((n_sub * n_tiles, 128), -1.0, dtype=np.float32)
        for sbi, (li, sg) in enumerate(per_core[c]):
            n = li.shape[0]
            cap = tiles_sb[sbi] * 128
            ip = np.zeros(cap, dtype=np.int16)
            ip[:n] = li
            sp = np.full(cap, -1.0, dtype=np.float32)
            sp[:n] = sg
            wrapped = ip.reshape(cap // 16, 16).T  # [16, cap//16]
            idx_arr[:, sbi * c16 : sbi * c16 + cap // 16] = (
                np.tile(wrapped, (8, 1))
            )
            seg_arr[sbi * n_tiles : sbi * n_tiles + tiles_sb[sbi]] = (
                sp.reshape(tiles_sb[sbi], 128)
            )
        fblob = np.zeros((128, F32_COLS), dtype=np.float32)
        fblob[:, :128] = np.eye(128, dtype=np.float32)
        fblob[:, O_WHID:O_WHID + H] = w_hid_f
        fblob[:, O_BHID] = b_hid_f
        fblob[:, O_WOUT] = w_out_f
        for g in range(N_GROUPS):
            gstart, gsize = GROUP_STARTS[g], GROUP_SEGS[g]
            fblob[:gsize, O_RECIP + g] = recip_all[
                c * SEGS_PER_CORE + gstart : c * SEGS_PER_CORE + gstart + gsize
            ]
            fblob[gsize:, O_RECIP + g] = 1.0
        fblob[0, O_Y:O_Y + SEGS_PER_CORE] = y_true[
            c * SEGS_PER_CORE : (c + 1) * SEGS_PER_CORE
        ]
        fblob[0, O_BOUT] = b_out_f[0]
        bblob = np.zeros((128, BF16_COLS), dtype=BF16_NP)
        bblob[:, :128] = iota.astype(BF16_NP)
        bblob[:, 128:] = seg_arr.T.astype(BF16_NP)
        in_maps.append({
            "embed": embed_bf16,
            "idx": idx_arr,
            "fblob": fblob,
            "bblob": bblob,
        })
    return c_sub, tiles_sb, in_maps


_CACHE = {}


def _get_nc(c_sub, tiles_sb):
    key = (c_sub, tiles_sb)
    nc = _CACHE.get(key)
    if nc is None:
        nc = bacc.Bacc("TRN2", target_bir_lowering=False, debug=False,
                       num_devices=N_CORES, num_swdge_queues=N_QUARTERS)
        _build(nc, c_sub, tiles_sb)
        _CACHE[key] = nc
    return nc


def kernel(token_ids, segment_ids, y_true, embed_table, W_hid, b_hid, W_out,
           b_out, _trace=False, _trace_kwargs=None):
    c_sub, tiles_sb, in_maps = _prep_inputs(token_ids, segment_ids, y_true,
                                            embed_table, W_hid, b_hid, W_out,
                                            b_out)
    nc = _get_nc(c_sub, tiles_sb)
    res = run_bass_kernel_spmd(nc, in_maps, core_ids=list(range(N_CORES)),
                               trace=_trace, **(_trace_kwargs or {}))
    total = np.float64(0.0)
    for r in res.results:
        total += np.float64(r["out"][0, 0])
    out = np.array(np.float32(total))
    if _trace:
        return out, res
    return out


# revision 16
# speedup vs baseline: 1.0483x; 1.0483x over previous
"""Trainium2 Bass kernel for the DAN classifier (gather + segment-mean + MLP + BCE).

Data-parallel across 8 NeuronCores: each core owns 512 whole sentences
(segments). Host splits the token stream at sentence boundaries, buckets each
core's tokens by (segment-group of 128, vocab-slice) so embedding row indices
fit in int16 for the bulk dma_gather, and pads each bucket to a common
capacity. On device: dma_gather (4 parallel SWDGE queues, one per vocab
slice - each queue's descriptors are generated by a different pair of Q7
cores) pulls bf16 embedding rows from HBM; a one-hot(segment) matmul on the
TensorEngine accumulates per-segment sums in fp32 PSUM; the tiny MLP head +
BCE run on-chip; each core emits its partial loss. Host sums the 8 partials
(the all-reduce of the scalar loss).

Key tunings over the first working version (guided by perfetto traces):
- vocab slices are uneven: queue 0 (whose gather instructions carry a fixed
  extra ~6-7us of drain/overhead per instruction) gets a smaller share.
- gather sizes are quantized per group for queues 1-3 so consecutive gather
  instructions share the num_idxs register (register reuse forced ~5us
  dispatch stalls between groups).
- BCE tail uses Softplus directly (loss = sum sp(x) - y*x), removing the
  Exp/Ln activations and a 1.5us activation-table load from the critical
  path; the y*x reduction is a single fused scalar_tensor_tensor.
- the last (small) group's one-hots are prebuilt during the gather phase.
- constant/metadata loads are consolidated into 3 DMAs (int16 idx, bf16
  blob, fp32 blob) instead of ~19 serial ~700ns HWDGE transfers.
"""

import sys

try:
    import concourse  # noqa: F401
except ImportError:
    sys.path.insert(0, "/opt/trn_rl_repo")

import ml_dtypes
import numpy as np

import concourse.tile as tile
from concourse import bacc, mybir
from concourse.bass_utils import run_bass_kernel_spmd

# Problem constants (hardcoded per harness contract).
V = 100000
H = 128
B = 4096
T = 409600
N_CORES = 8

SEGS_PER_CORE = B // N_CORES          # 512
# Tapered segment groups: early groups big (pipeline fill), last group small
# so the final gather + matmul + MLP tail is short.
GROUP_SEGS = (128, 128, 128, 96, 32)
GROUP_STARTS = (0, 128, 256, 384, 480)
N_GROUPS = len(GROUP_SEGS)
N_QUARTERS = 4                        # vocab slices / SWDGE queues
# Uneven vocab slices: queue 0 smaller (fixed per-instruction overhead there),
# queues 1-3 equal. All slice sizes must stay < 32768 for int16 indices.
QB = (0, 20608, 47072, 73536, 100000)
QSIZES = tuple(QB[i + 1] - QB[i] for i in range(N_QUARTERS))

F32 = mybir.dt.float32
BF16 = mybir.dt.bfloat16
I16 = mybir.dt.int16
BF16_NP = ml_dtypes.bfloat16


def _build(nc, c_sub, tiles_sb):
    """Emit the SPMD per-core graph. c_sub = padded tokens per sub-block."""
    n_tiles = c_sub // 128  # token tiles of 128 per sub-block
    n_sub = N_GROUPS * N_QUARTERS
    c16 = c_sub // 16

    # fp32 blob layout (columns): ident(128) | w_hid(128) | b_hid(1) |
    # w_out(1) | recip(N_GROUPS) | y(SEGS_PER_CORE, row 0) | b_out(1, row 0)
    O_IDENT = 0
    O_WHID = O_IDENT + 128
    O_BHID = O_WHID + H
    O_WOUT = O_BHID + 1
    O_RECIP = O_WOUT + 1
    O_Y = O_RECIP + N_GROUPS
    O_BOUT = O_Y + SEGS_PER_CORE
    F32_COLS = O_BOUT + 1

    # bf16 blob layout: iota(128) | seg(n_sub * n_tiles)
    O_IOTA = 0
    O_SEG = O_IOTA + 128
    BF16_COLS = O_SEG + n_sub * n_tiles

    # ---- DRAM parameters (per-core shards arrive via in_maps) ----
    embed = nc.dram_tensor("embed", [V, H], BF16, kind="ExternalInput")
    idx_d = nc.dram_tensor("idx", [128, n_sub * c16], I16, kind="ExternalInput")
    fblob_d = nc.dram_tensor("fblob", [128, F32_COLS], F32, kind="ExternalInput")
    bblob_d = nc.dram_tensor("bblob", [128, BF16_COLS], BF16,
                             kind="ExternalInput")
    out_d = nc.dram_tensor("out", [1, 1], F32, kind="ExternalOutput")

    with tile.TileContext(nc) as tc:
        with (
            tc.tile_pool(name="const", bufs=1) as cpool,
            tc.tile_pool(name="gather", bufs=4) as gpool,
            tc.tile_pool(name="onehot", bufs=4) as opool,
            tc.tile_pool(name="mlp", bufs=1) as mpool,
            tc.tile_pool(name="psum", bufs=2, space="PSUM") as ppool,
            tc.tile_pool(name="psum_mlp", bufs=1, space="PSUM") as pmpool,
        ):
            # ---- warm activation tables first ----
            warm = cpool.tile([1, 1], F32)
            for fn in (mybir.ActivationFunctionType.Tanh,
                       mybir.ActivationFunctionType.Exp,
                       mybir.ActivationFunctionType.Ln):
                nc.scalar.activation(out=warm[:], in_=warm[:], func=fn)

            # ---- consolidated constants / metadata loads ----
            idx_sb = cpool.tile([128, n_sub * c16], I16)
            # group 0's idx slice first (gates the first gathers), rest after
            g0q = N_QUARTERS * c16
            nc.sync.dma_start(out=idx_sb[:, :g0q], in_=idx_d[:, :g0q])
            nc.sync.dma_start(out=idx_sb[:, g0q:], in_=idx_d[:, g0q:])
            fblob = cpool.tile([128, F32_COLS], F32)
            nc.sync.dma_start(out=fblob[:], in_=fblob_d[:])
            bblob = cpool.tile([128, BF16_COLS], BF16)
            nc.sync.dma_start(out=bblob[:], in_=bblob_d[:])

            ident_sb = fblob[:, O_IDENT:O_IDENT + 128]
            w_hid_sb = fblob[:, O_WHID:O_WHID + H]
            b_hid_sb = fblob[:, O_BHID:O_BHID + 1]
            w_out_sb = fblob[:, O_WOUT:O_WOUT + 1]
            recip_sb = fblob[:, O_RECIP:O_RECIP + N_GROUPS]
            y_sb = fblob[0:1, O_Y:O_Y + SEGS_PER_CORE]
            b_out_sb = fblob[0:1, O_BOUT:O_BOUT + 1]
            iota_sb = bblob[:, O_IOTA:O_IOTA + 128]
            seg_sb = bblob[:, O_SEG:]

            sent_t = mpool.tile([128, SEGS_PER_CORE], F32)  # [H, seg]
            psum_hid = pmpool.tile([128, SEGS_PER_CORE], F32, tag="psum_hid")
            psum_p = pmpool.tile([1, SEGS_PER_CORE], F32, tag="psum_p")
            hid = mpool.tile([128, SEGS_PER_CORE], F32)

            # ---- prebuild the last two groups' one-hots (persistent tiles;
            # they would otherwise sit on the critical tail waiting for
            # onehot-pool buffers chained to earlier groups' gathers) ----
            prebuilt = {}
            for g in (N_GROUPS - 2, N_GROUPS - 1):
                for q in range(N_QUARTERS):
                    sb = g * N_QUARTERS + q
                    tsb = tiles_sb[sb]
                    oh = cpool.tile([128, tsb, 128], BF16,
                                    name=f"ohp_{g}_{q}")
                    nc.vector.tensor_tensor(
                        out=oh[:],
                        in0=seg_sb[:, sb * n_tiles : sb * n_tiles + tsb]
                        .rearrange("p (t u) -> p t u", u=1)
                        .to_broadcast([128, tsb, 128]),
                        in1=iota_sb
                        .rearrange("p (u m) -> p u m", u=1)
                        .to_broadcast([128, tsb, 128]),
                        op=mybir.AluOpType.is_equal,
                    )
                    prebuilt[sb] = oh

            for g in range(N_GROUPS):
                gt = gpool.tile([128, N_QUARTERS, n_tiles, 128], BF16,
                                tag="gather")
                for q in (1, 2, 3, 0):
                    sb = g * N_QUARTERS + q
                    nidx = tiles_sb[sb] * 128
                    nc.gpsimd.dma_gather(
                        gt[:, q, 0:tiles_sb[sb], :],
                        embed[QB[q] : QB[q] + QSIZES[q], :],
                        idx_sb[:, sb * c16 : sb * c16 + nidx // 16],
                        nidx,
                        nidx,
                        H,
                        single_packet=False,
                        queue_num=q,
                    )

                psum_s = ppool.tile([128, H], F32, tag="psum_s")
                for q in range(N_QUARTERS):
                    sb = g * N_QUARTERS + q
                    tsb = tiles_sb[sb]
                    if sb in prebuilt:
                        oh = prebuilt[sb]
                    else:
                        oh = opool.tile([128, n_tiles, 128], BF16, tag="onehot")
                        nc.vector.tensor_tensor(
                            out=oh[:, :tsb, :],
                            in0=seg_sb[:, sb * n_tiles : sb * n_tiles + tsb]
                            .rearrange("p (t u) -> p t u", u=1)
                            .to_broadcast([128, tsb, 128]),
                            in1=iota_sb
                            .rearrange("p (u m) -> p u m", u=1)
                            .to_broadcast([128, tsb, 128]),
                            op=mybir.AluOpType.is_equal,
                        )
                    for j in range(tsb):
                        nc.tensor.matmul(
                            psum_s[:],
                            lhsT=oh[:, j, :],
                            rhs=gt[:, q, j, :],
                            start=(q == 0 and j == 0),
                            stop=(q == N_QUARTERS - 1 and j == tsb - 1),
                        )

                # segment means for this group: psum * (1/count)
                gstart, gsize = GROUP_STARTS[g], GROUP_SEGS[g]
                sent_g = mpool.tile([128, H], F32, tag="sent_g")
                nc.vector.tensor_scalar(
                    out=sent_g[:],
                    in0=psum_s[:],
                    scalar1=recip_sb[:, g : g + 1],
                    scalar2=None,
                    op0=mybir.AluOpType.mult,
                )
                # transpose [seg, H] -> [H, seg] chunk of sent_t
                psum_t = ppool.tile([128, 128], F32, tag="psum_t")
                nc.tensor.transpose(psum_t[:], sent_g[:], ident_sb)
                nc.vector.tensor_copy(
                    out=sent_t[:, gstart : gstart + gsize],
                    in_=psum_t[:, :gsize],
                )
                nc.tensor.matmul(psum_hid[:, gstart : gstart + gsize],
                                 lhsT=w_hid_sb,
                                 rhs=sent_t[:, gstart : gstart + gsize],
                                 start=True, stop=True)
                nc.scalar.activation(
                    out=hid[:, gstart : gstart + gsize],
                    in_=psum_hid[:, gstart : gstart + gsize],
                    func=mybir.ActivationFunctionType.Tanh,
                    bias=b_hid_sb,
                )
                nc.tensor.matmul(psum_p[:, gstart : gstart + gsize],
                                 lhsT=w_out_sb,
                                 rhs=hid[:, gstart : gstart + gsize],
                                 start=True, stop=True)

            # ---- BCE tail ----
            # x = W_out^T hid + b_out ;  loss = sum softplus(x) - sum y*x
            # softplus(x) = ln(1 + exp(x)) via the Exp and Ln tables.
            ep = mpool.tile([1, SEGS_PER_CORE], F32)
            nc.scalar.activation(
                out=ep[:], in_=psum_p[:],
                func=mybir.ActivationFunctionType.Exp,
                bias=b_out_sb,
            )
            sp = mpool.tile([1, SEGS_PER_CORE], F32)
            sp_sum = mpool.tile([1, 1], F32)
            nc.scalar.activation(
                out=sp[:], in_=ep[:],
                func=mybir.ActivationFunctionType.Ln,
                bias=1.0, accum_out=sp_sum[:],
            )
            yx = mpool.tile([1, SEGS_PER_CORE], F32)
            yx_sum = mpool.tile([1, 1], F32)
            nc.vector.scalar_tensor_tensor(
                out=yx[:], in0=psum_p[:], scalar=b_out_sb, in1=y_sb,
                op0=mybir.AluOpType.add, op1=mybir.AluOpType.mult,
                accum_out=yx_sum[:],
            )
            loss = mpool.tile([1, 1], F32)
            nc.vector.tensor_tensor(out=loss[:], in0=sp_sum[:], in1=yx_sum[:],
                                    op=mybir.AluOpType.subtract)
            nc.sync.dma_start(out=out_d[:], in_=loss[:])

    nc.compile()
    return nc


def _prep_inputs(token_ids, segment_ids, y_true, embed_table, W_hid, b_hid,
                 W_out, b_out):
    """Host-side shard + bucket + pad. Returns (c_sub, tiles_sb, in_maps)."""
    token_ids = np.asarray(token_ids, dtype=np.int64)
    segment_ids = np.asarray(segment_ids, dtype=np.int64)
    y_true = np.asarray(y_true, dtype=np.float32)
    embed_bf16 = np.ascontiguousarray(
        np.asarray(embed_table, dtype=np.float32).astype(BF16_NP))

    # sentence-aligned core boundaries
    bounds = np.searchsorted(segment_ids, np.arange(0, B + 1, SEGS_PER_CORE))
    counts = np.bincount(segment_ids, minlength=B).astype(np.float32)
    recip_all = 1.0 / np.maximum(counts, 1.0)

    qb_arr = np.asarray(QB[1:-1], dtype=np.int64)

    # bucket tokens per (core, group, vocab-slice)
    per_core = []
    for c in range(N_CORES):
        lo, hi = bounds[c], bounds[c + 1]
        tid = token_ids[lo:hi]
        seg_loc = segment_ids[lo:hi] - c * SEGS_PER_CORE
        starts = np.asarray(GROUP_STARTS, dtype=np.int64)
        grp = np.searchsorted(starts[1:], seg_loc, side="right")
        seg_in_grp = (seg_loc - starts[grp]).astype(np.float32)
        q = np.searchsorted(qb_arr, tid, side="right")
        loc_idx = (tid - np.asarray(QB, dtype=np.int64)[q]).astype(np.int64)
        subs = []
        for g in range(N_GROUPS):
            for qq in range(N_QUARTERS):
                sel = (grp == g) & (q == qq)
                li, sg = loc_idx[sel], seg_in_grp[sel]
                order = np.argsort(li, kind="stable")
                subs.append((li[order], sg[order]))
        per_core.append(subs)

    n_sub = N_GROUPS * N_QUARTERS
    sb_max = [0] * n_sub
    for c in range(N_CORES):
        for sbi, (li, sg) in enumerate(per_core[c]):
            sb_max[sbi] = max(sb_max[sbi], li.shape[0])
    # Quantize gather sizes: queues 1-3 share one tile count per group (so
    # consecutive gather instructions reuse the same num_idxs register);
    # queue 0 gets its own (smaller) count per group.
    tiles_raw = [(m + 127) // 128 for m in sb_max]
    tiles_sb = [0] * n_sub
    for g in range(N_GROUPS):
        t123 = max(tiles_raw[g * N_QUARTERS + q] for q in (1, 2, 3))
        for q in range(N_QUARTERS):
            sb = g * N_QUARTERS + q
            tiles_sb[sb] = tiles_raw[sb] if q == 0 else t123
    tiles_sb = tuple(tiles_sb)
    c_sub = 128 * max(tiles_sb)
    n_tiles = c_sub // 128
    c16 = c_sub // 16

    # fp32 / bf16 blob column offsets (keep in lockstep with _build)
    O_WHID = 128
    O_BHID = O_WHID + H
    O_WOUT = O_BHID + 1
    O_RECIP = O_WOUT + 1
    O_Y = O_RECIP + N_GROUPS
    O_BOUT = O_Y + SEGS_PER_CORE
    F32_COLS = O_BOUT + 1
    BF16_COLS = 128 + n_sub * n_tiles

    iota = np.broadcast_to(np.arange(128, dtype=np.float32), (128, 128))
    w_hid_f = np.ascontiguousarray(np.asarray(W_hid, dtype=np.float32))
    b_hid_f = np.asarray(b_hid, dtype=np.float32).reshape(H)
    w_out_f = np.asarray(W_out, dtype=np.float32).reshape(H)
    b_out_f = np.asarray(b_out, dtype=np.float32).reshape(1)

    in_maps = []
    for c in range(N_CORES):
        idx_arr = np.zeros((128, n_sub * c16), dtype=np.int16)
        seg_arr = np.full((n_sub * n_tiles, 128), -1.0, dtype=np.float32)
        for sbi, (li, sg) in enumerate(per_core[c]):
            n = li.shape[0]
            cap = tiles_sb[sbi] * 128
            ip = np.zeros(cap, dtype=np.int16)
            ip[:n] = li
            sp = np.full(cap, -1.0, dtype=np.float32)
            sp[:n] = sg
            wrapped = ip.reshape(cap // 16, 16).T  # [16, cap//16]
            idx_arr[:, sbi * c16 : sbi * c16 + cap // 16] = (
                np.tile(wrapped, (8, 1))
            )
            seg_arr[sbi * n_tiles : sbi * n_tiles + tiles_sb[sbi]] = (
                sp.reshape(tiles_sb[sbi], 128)
            )
        fblob = np.zeros((128, F32_COLS), dtype=np.float32)
        fblob[:, :128] = np.eye(128, dtype=np.float32)
        fblob[:, O_WHID:O_WHID + H] = w_hid_f
        fblob[:, O_BHID] = b_hid_f
        fblob[:, O_WOUT] = w_out_f
        for g in range(N_GROUPS):
            gstart, gsize = GROUP_STARTS[g], GROUP_SEGS[g]
            fblob[:gsize, O_RECIP + g] = recip_all[
                c * SEGS_PER_CORE + gstart : c * SEGS_PER_CORE + gstart + gsize
            ]
            fblob[gsize:, O_RECIP + g] = 1.0
        fblob[0, O_Y:O_Y + SEGS_PER_CORE] = y_true[
            c * SEGS_PER_CORE : (c + 1) * SEGS_PER_CORE
        ]
        fblob[0, O_BOUT] = b_out_f[0]
        bblob = np.zeros((128, BF16_COLS), dtype=BF16_NP)
        bblob[:, :128] = iota.astype(BF16_NP)
        bblob[:, 128:] = seg_arr.T.astype(BF16_NP)
        in_maps.append({
            "embed": embed_bf16,
            "idx": idx_arr,
            "fblob": fblob,
            "bblob": bblob,
        })
    return c_sub, tiles_sb, in_maps


_CACHE = {}


def _get_nc(c_sub, tiles_sb):
    key = (c_sub, tiles_sb)
    nc = _CACHE.get(key)
    if nc is None:
        nc = bacc.Bacc("TRN2", target_bir_lowering=False, debug=False,
                       num_devices=N_CORES, num_swdge_queues=N_QUARTERS)
        _build(nc, c_sub, tiles_sb)
        _CACHE[key] = nc
    return nc


def kernel(token_ids, segment_ids, y_true, embed_table, W_hid, b_hid, W_out,
           b_out, _trace=False, _trace_kwargs=None):
    c_sub, tiles_sb, in_maps = _prep_inputs(token_ids, segment_ids, y_true,
                                            embed_table, W_hid, b_hid, W_out,
                                            b_out)
    nc = _get_nc(c_sub, tiles_sb)
    res = run_bass_kernel_spmd(nc, in_maps, core_ids=list(range(N_CORES)),
                               trace=_trace, **(_trace_kwargs or {}))
    total = np.float64(0.0)
    for r in res.results:
        total += np.float64(r["out"][0, 0])
    out = np.array(np.float32(total))
    if _trace:
        return out, res
    return out


# revision 22
# speedup vs baseline: 1.0486x; 1.0003x over previous
"""Trainium2 Bass kernel for the DAN classifier (gather + segment-mean + MLP + BCE).

Data-parallel across 8 NeuronCores: each core owns 512 whole sentences
(segments). Host splits the token stream at sentence boundaries, buckets each
core's tokens by (segment-group of 128, vocab-slice) so embedding row indices
fit in int16 for the bulk dma_gather, and pads each bucket to a common
capacity. On device: dma_gather (4 parallel SWDGE queues, one per vocab
slice - each queue's descriptors are generated by a different pair of Q7
cores) pulls bf16 embedding rows from HBM; a one-hot(segment) matmul on the
TensorEngine accumulates per-segment sums in fp32 PSUM; the tiny MLP head +
BCE run on-chip; each core emits its partial loss. Host sums the 8 partials
(the all-reduce of the scalar loss).

Key tunings over the first working version (guided by perfetto traces):
- vocab slices are uneven: queue 0 (whose gather instructions carry a fixed
  extra ~6-7us of drain/overhead per instruction) gets a smaller share.
- gather sizes are quantized per group for queues 1-3 so consecutive gather
  instructions share the num_idxs register (register reuse forced ~5us
  dispatch stalls between groups).
- BCE tail uses Softplus directly (loss = sum sp(x) - y*x), removing the
  Exp/Ln activations and a 1.5us activation-table load from the critical
  path; the y*x reduction is a single fused scalar_tensor_tensor.
- the last (small) group's one-hots are prebuilt during the gather phase.
- constant/metadata loads are consolidated into 3 DMAs (int16 idx, bf16
  blob, fp32 blob) instead of ~19 serial ~700ns HWDGE transfers.
"""

import sys

try:
    import concourse  # noqa: F401
except ImportError:
    sys.path.insert(0, "/opt/trn_rl_repo")

import ml_dtypes
import numpy as np

import concourse.tile as tile
from concourse import bacc, mybir
from concourse.bass_utils import run_bass_kernel_spmd

# Problem constants (hardcoded per harness contract).
V = 100000
H = 128
B = 4096
T = 409600
N_CORES = 8

SEGS_PER_CORE = B // N_CORES          # 512
# Tapered segment groups: early groups big (pipeline fill), last group small
# so the final gather + matmul + MLP tail is short.
GROUP_SEGS = (128, 128, 128, 112, 16)
GROUP_STARTS = (0, 128, 256, 384, 496)
N_GROUPS = len(GROUP_SEGS)
N_QUARTERS = 4                        # vocab slices / SWDGE queues
# Uneven vocab slices: queue 0 smaller (fixed per-instruction overhead there),
# queues 1-3 equal. All slice sizes must stay < 32768 for int16 indices.
QB = (0, 20608, 47072, 73536, 100000)
QSIZES = tuple(QB[i + 1] - QB[i] for i in range(N_QUARTERS))

F32 = mybir.dt.float32
BF16 = mybir.dt.bfloat16
I16 = mybir.dt.int16
BF16_NP = ml_dtypes.bfloat16


def _build(nc, c_sub, tiles_sb):
    """Emit the SPMD per-core graph. c_sub = padded tokens per sub-block."""
    n_tiles = c_sub // 128  # token tiles of 128 per sub-block
    n_sub = N_GROUPS * N_QUARTERS
    c16 = c_sub // 16

    # fp32 blob layout (columns): w_hid(128) | b_hid(1) | w_out(1) |
    # recip replicated across partitions (N_GROUPS*128) | y(row 0) | b_out
    O_WHID = 0
    O_BHID = O_WHID + H
    O_WOUT = O_BHID + 1
    O_RREP = O_WOUT + 1
    O_Y = O_RREP + N_GROUPS * 128
    O_BOUT = O_Y + SEGS_PER_CORE
    F32_COLS = O_BOUT + 1

    # bf16 blob layout: iota(128) | seg(n_sub * n_tiles)
    O_IOTA = 0
    O_SEG = O_IOTA + 128
    BF16_COLS = O_SEG + n_sub * n_tiles

    # ---- DRAM parameters (per-core shards arrive via in_maps) ----
    embed = nc.dram_tensor("embed", [V, H], BF16, kind="ExternalInput")
    idx_d = nc.dram_tensor("idx", [128, n_sub * c16], I16, kind="ExternalInput")
    fblob_d = nc.dram_tensor("fblob", [128, F32_COLS], F32, kind="ExternalInput")
    bblob_d = nc.dram_tensor("bblob", [128, BF16_COLS], BF16,
                             kind="ExternalInput")
    out_d = nc.dram_tensor("out", [1, 1], F32, kind="ExternalOutput")

    with tile.TileContext(nc) as tc:
        with (
            tc.tile_pool(name="const", bufs=1) as cpool,
            tc.tile_pool(name="gather", bufs=4) as gpool,
            tc.tile_pool(name="onehot", bufs=4) as opool,
            tc.tile_pool(name="mlp", bufs=1) as mpool,
            tc.tile_pool(name="psum", bufs=2, space="PSUM") as ppool,
            tc.tile_pool(name="psum_mlp", bufs=1, space="PSUM") as pmpool,
        ):
            # ---- warm activation tables first ----
            warm = cpool.tile([1, 1], F32)
            for fn in (mybir.ActivationFunctionType.Tanh,
                       mybir.ActivationFunctionType.Exp,
                       mybir.ActivationFunctionType.Ln):
                nc.scalar.activation(out=warm[:], in_=warm[:], func=fn)

            # ---- consolidated constants / metadata loads ----
            idx_sb = cpool.tile([128, n_sub * c16], I16)
            # group 0's idx slice first (gates the first gathers), rest after
            g0q = N_QUARTERS * c16
            nc.sync.dma_start(out=idx_sb[:, :g0q], in_=idx_d[:, :g0q])
            nc.sync.dma_start(out=idx_sb[:, g0q:], in_=idx_d[:, g0q:])
            fblob = cpool.tile([128, F32_COLS], F32)
            nc.sync.dma_start(out=fblob[:], in_=fblob_d[:])
            bblob = cpool.tile([128, BF16_COLS], BF16)
            nc.sync.dma_start(out=bblob[:], in_=bblob_d[:])

            w_hid_sb = fblob[:, O_WHID:O_WHID + H]
            b_hid_sb = fblob[:, O_BHID:O_BHID + 1]
            w_out_sb = fblob[:, O_WOUT:O_WOUT + 1]
            rrep_sb = fblob[:, O_RREP:O_RREP + N_GROUPS * 128]
            y_sb = fblob[0:1, O_Y:O_Y + SEGS_PER_CORE]
            b_out_sb = fblob[0:1, O_BOUT:O_BOUT + 1]
            iota_sb = bblob[:, O_IOTA:O_IOTA + 128]
            seg_sb = bblob[:, O_SEG:]

            sent_t = mpool.tile([128, SEGS_PER_CORE], F32)  # [H, seg]
            psum_hid = pmpool.tile([128, SEGS_PER_CORE], F32, tag="psum_hid")
            psum_p = pmpool.tile([1, SEGS_PER_CORE], F32, tag="psum_p")
            hid = mpool.tile([128, SEGS_PER_CORE], F32)

            # ---- prebuild the last two groups' one-hots (persistent tiles;
            # they would otherwise sit on the critical tail waiting for
            # onehot-pool buffers chained to earlier groups' gathers) ----
            prebuilt = {}
            for g in (N_GROUPS - 2, N_GROUPS - 1):
                for q in range(N_QUARTERS):
                    sb = g * N_QUARTERS + q
                    tsb = tiles_sb[sb]
                    oh = cpool.tile([128, tsb, 128], BF16,
                                    name=f"ohp_{g}_{q}")
                    nc.vector.tensor_tensor(
                        out=oh[:],
                        in0=seg_sb[:, sb * n_tiles : sb * n_tiles + tsb]
                        .rearrange("p (t u) -> p t u", u=1)
                        .to_broadcast([128, tsb, 128]),
                        in1=iota_sb
                        .rearrange("p (u m) -> p u m", u=1)
                        .to_broadcast([128, tsb, 128]),
                        op=mybir.AluOpType.is_equal,
                    )
                    prebuilt[sb] = oh

            for g in range(N_GROUPS):
                gt = gpool.tile([128, N_QUARTERS, n_tiles, 128], BF16,
                                tag="gather")
                for q in (1, 2, 3, 0):
                    sb = g * N_QUARTERS + q
                    nidx = tiles_sb[sb] * 128
                    nc.gpsimd.dma_gather(
                        gt[:, q, 0:tiles_sb[sb], :],
                        embed[QB[q] : QB[q] + QSIZES[q], :],
                        idx_sb[:, sb * c16 : sb * c16 + nidx // 16],
                        nidx,
                        nidx,
                        H,
                        single_packet=False,
                        queue_num=q,
                    )

                # lhsT = gathered rows (weights), rhs = one-hot: psum_s comes
                # out already transposed as [H, seg-in-group], so no PE
                # transpose / identity matmul is needed downstream.
                psum_s = ppool.tile([128, 128], F32, tag="psum_s")
                for q in range(N_QUARTERS):
                    sb = g * N_QUARTERS + q
                    tsb = tiles_sb[sb]
                    if sb in prebuilt:
                        oh = prebuilt[sb]
                    else:
                        oh = opool.tile([128, n_tiles, 128], BF16, tag="onehot")
                        nc.vector.tensor_tensor(
                            out=oh[:, :tsb, :],
                            in0=seg_sb[:, sb * n_tiles : sb * n_tiles + tsb]
                            .rearrange("p (t u) -> p t u", u=1)
                            .to_broadcast([128, tsb, 128]),
                            in1=iota_sb
                            .rearrange("p (u m) -> p u m", u=1)
                            .to_broadcast([128, tsb, 128]),
                            op=mybir.AluOpType.is_equal,
                        )
                    for j in range(tsb):
                        nc.tensor.matmul(
                            psum_s[:],
                            lhsT=gt[:, q, j, :],
                            rhs=oh[:, j, :],
                            start=(q == 0 and j == 0),
                            stop=(q == N_QUARTERS - 1 and j == tsb - 1),
                        )

                # segment means: evacuate PSUM and scale by 1/count in one op
                gstart, gsize = GROUP_STARTS[g], GROUP_SEGS[g]
                nc.vector.tensor_tensor(
                    out=sent_t[:, gstart : gstart + gsize],
                    in0=psum_s[:, :gsize],
                    in1=rrep_sb[:, g * 128 : g * 128 + gsize],
                    op=mybir.AluOpType.mult,
                )
                nc.tensor.matmul(psum_hid[:, gstart : gstart + gsize],
                                 lhsT=w_hid_sb,
                                 rhs=sent_t[:, gstart : gstart + gsize],
                                 start=True, stop=True)
                nc.scalar.activation(
                    out=hid[:, gstart : gstart + gsize],
                    in_=psum_hid[:, gstart : gstart + gsize],
                    func=mybir.ActivationFunctionType.Tanh,
                    bias=b_hid_sb,
                )
                nc.tensor.matmul(psum_p[:, gstart : gstart + gsize],
                                 lhsT=w_out_sb,
                                 rhs=hid[:, gstart : gstart + gsize],
                                 start=True, stop=True)

            # ---- BCE tail ----
            # x = W_out^T hid + b_out ;  loss = sum softplus(x) - sum y*x
            # softplus(x) = ln(1 + exp(x)) via the Exp and Ln tables.
            ep = mpool.tile([1, SEGS_PER_CORE], F32)
            nc.scalar.activation(
                out=ep[:], in_=psum_p[:],
                func=mybir.ActivationFunctionType.Exp,
                bias=b_out_sb,
            )
            sp = mpool.tile([1, SEGS_PER_CORE], F32)
            sp_sum = mpool.tile([1, 1], F32)
            nc.scalar.activation(
                out=sp[:], in_=ep[:],
                func=mybir.ActivationFunctionType.Ln,
                bias=1.0, accum_out=sp_sum[:],
            )
            yx = mpool.tile([1, SEGS_PER_CORE], F32)
            yx_sum = mpool.tile([1, 1], F32)
            nc.vector.scalar_tensor_tensor(
                out=yx[:], in0=psum_p[:], scalar=b_out_sb, in1=y_sb,
                op0=mybir.AluOpType.add, op1=mybir.AluOpType.mult,
                accum_out=yx_sum[:],
            )
            loss = mpool.tile([1, 1], F32)
            nc.vector.tensor_tensor(out=loss[:], in0=sp_sum[:], in1=yx_sum[:],
                                    op=mybir.AluOpType.subtract)
            nc.sync.dma_start(out=out_d[:], in_=loss[:])

    nc.compile()
    return nc


def _prep_inputs(token_ids, segment_ids, y_true, embed_table, W_hid, b_hid,
                 W_out, b_out):
    """Host-side shard + bucket + pad. Returns (c_sub, tiles_sb, in_maps)."""
    token_ids = np.asarray(token_ids, dtype=np.int64)
    segment_ids = np.asarray(segment_ids, dtype=np.int64)
    y_true = np.asarray(y_true, dtype=np.float32)
    embed_bf16 = np.ascontiguousarray(
        np.asarray(embed_table, dtype=np.float32).astype(BF16_NP))

    # sentence-aligned core boundaries
    bounds = np.searchsorted(segment_ids, np.arange(0, B + 1, SEGS_PER_CORE))
    counts = np.bincount(segment_ids, minlength=B).astype(np.float32)
    recip_all = 1.0 / np.maximum(counts, 1.0)

    qb_arr = np.asarray(QB[1:-1], dtype=np.int64)

    # bucket tokens per (core, group, vocab-slice)
    per_core = []
    for c in range(N_CORES):
        lo, hi = bounds[c], bounds[c + 1]
        tid = token_ids[lo:hi]
        seg_loc = segment_ids[lo:hi] - c * SEGS_PER_CORE
        starts = np.asarray(GROUP_STARTS, dtype=np.int64)
        grp = np.searchsorted(starts[1:], seg_loc, side="right")
        seg_in_grp = (seg_loc - starts[grp]).astype(np.float32)
        q = np.searchsorted(qb_arr, tid, side="right")
        loc_idx = (tid - np.asarray(QB, dtype=np.int64)[q]).astype(np.int64)
        subs = []
        for g in range(N_GROUPS):
            for qq in range(N_QUARTERS):
                sel = (grp == g) & (q == qq)
                li, sg = loc_idx[sel], seg_in_grp[sel]
                order = np.argsort(li, kind="stable")
                subs.append((li[order], sg[order]))
        per_core.append(subs)

    n_sub = N_GROUPS * N_QUARTERS
    sb_max = [0] * n_sub
    for c in range(N_CORES):
        for sbi, (li, sg) in enumerate(per_core[c]):
            sb_max[sbi] = max(sb_max[sbi], li.shape[0])
    # Quantize gather sizes: queues 1-3 share one tile count per group (so
    # consecutive gather instructions reuse the same num_idxs register);
    # queue 0 gets its own (smaller) count per group.
    tiles_raw = [(m + 127) // 128 for m in sb_max]
    tiles_sb = [0] * n_sub
    for g in range(N_GROUPS):
        t123 = max(tiles_raw[g * N_QUARTERS + q] for q in (1, 2, 3))
        for q in range(N_QUARTERS):
            sb = g * N_QUARTERS + q
            tiles_sb[sb] = tiles_raw[sb] if q == 0 else t123
    tiles_sb = tuple(tiles_sb)
    c_sub = 128 * max(tiles_sb)
    n_tiles = c_sub // 128
    c16 = c_sub // 16

    # fp32 / bf16 blob column offsets (keep in lockstep with _build)
    O_WHID = 0
    O_BHID = O_WHID + H
    O_WOUT = O_BHID + 1
    O_RREP = O_WOUT + 1
    O_Y = O_RREP + N_GROUPS * 128
    O_BOUT = O_Y + SEGS_PER_CORE
    F32_COLS = O_BOUT + 1
    BF16_COLS = 128 + n_sub * n_tiles

    iota = np.broadcast_to(np.arange(128, dtype=np.float32), (128, 128))
    w_hid_f = np.ascontiguousarray(np.asarray(W_hid, dtype=np.float32))
    b_hid_f = np.asarray(b_hid, dtype=np.float32).reshape(H)
    w_out_f = np.asarray(W_out, dtype=np.float32).reshape(H)
    b_out_f = np.asarray(b_out, dtype=np.float32).reshape(1)

    in_maps = []
    for c in range(N_CORES):
        idx_arr = np.zeros((128, n_sub * c16), dtype=np.int16)
        seg_arr = np.full((n_sub * n_tiles, 128), -1.0, dtype=np.float32)
        for sbi, (li, sg) in enumerate(per_core[c]):
            n = li.shape[0]
            cap = tiles_sb[sbi] * 128
            ip = np.zeros(cap, dtype=np.int16)
            ip[:n] = li
            sp = np.full(cap, -1.0, dtype=np.float32)
            sp[:n] = sg
            wrapped = ip.reshape(cap // 16, 16).T  # [16, cap//16]
            idx_arr[:, sbi * c16 : sbi * c16 + cap // 16] = (
                np.tile(wrapped, (8, 1))
            )
            seg_arr[sbi * n_tiles : sbi * n_tiles + tiles_sb[sbi]] = (
                sp.reshape(tiles_sb[sbi], 128)
            )
        fblob = np.zeros((128, F32_COLS), dtype=np.float32)
        fblob[:, O_WHID:O_WHID + H] = w_hid_f
        fblob[:, O_BHID] = b_hid_f
        fblob[:, O_WOUT] = w_out_f
        for g in range(N_GROUPS):
            gstart, gsize = GROUP_STARTS[g], GROUP_SEGS[g]
            rg = np.ones(128, dtype=np.float32)
            rg[:gsize] = recip_all[
                c * SEGS_PER_CORE + gstart : c * SEGS_PER_CORE + gstart + gsize
            ]
            fblob[:, O_RREP + g * 128 : O_RREP + (g + 1) * 128] = rg
        fblob[0, O_Y:O_Y + SEGS_PER_CORE] = y_true[
            c * SEGS_PER_CORE : (c + 1) * SEGS_PER_CORE
        ]
        fblob[0, O_BOUT] = b_out_f[0]
        bblob = np.zeros((128, BF16_COLS), dtype=BF16_NP)
        bblob[:, :128] = iota.astype(BF16_NP)
        bblob[:, 128:] = seg_arr.T.astype(BF16_NP)
        in_maps.append({
            "embed": embed_bf16,
            "idx": idx_arr,
            "fblob": fblob,
            "bblob": bblob,
        })
    return c_sub, tiles_sb, in_maps


_CACHE = {}


def _get_nc(c_sub, tiles_sb):
    key = (c_sub, tiles_sb)
    nc = _CACHE.get(key)
    if nc is None:
        nc = bacc.Bacc("TRN2", target_bir_lowering=False, debug=False,
                       num_devices=N_CORES, num_swdge_queues=N_QUARTERS)
        _build(nc, c_sub, tiles_sb)
        _CACHE[key] = nc
    return nc


def kernel(token_ids, segment_ids, y_true, embed_table, W_hid, b_hid, W_out,
           b_out, _trace=False, _trace_kwargs=None):
    c_sub, tiles_sb, in_maps = _prep_inputs(token_ids, segment_ids, y_true,
                                            embed_table, W_hid, b_hid, W_out,
                                            b_out)
    nc = _get_nc(c_sub, tiles_sb)
    res = run_bass_kernel_spmd(nc, in_maps, core_ids=list(range(N_CORES)),
                               trace=_trace, **(_trace_kwargs or {}))
    total = np.float64(0.0)
    for r in res.results:
        total += np.float64(r["out"][0, 0])
    out = np.array(np.float32(total))
    if _trace:
        return out, res
    return out
